# revision 8
# baseline (speedup 1.0000x reference)
"""Trainium2 Bass kernel for nn_ListenerModelBertAttCtxHist.

Data-parallel over the batch dim: 64 batches -> 8 NeuronCores x 8 batches.
Weights are replicated. All heavy compute (matmuls, masked softmax,
history mean, normalization, attention combine) runs on-device in a single
fused Bass/Tile kernel per core; the host only slices inputs per core and
lays them out (transpose/reshape) for upload.

Model (per batch b):
  input_reps = relu(X @ W_e2h + b_e2h)                  X: [S=512, E=768]
  ctx        = relu(vc @ W_ctx + b_ctx)                 vc: [12288]
  mm         = relu(input_reps @ Wmm_A + ctx @ Wmm_B + b_mm)
  scores     = tanh(mm @ W_a1 + b_a1) @ W_a2 (+b_a2, dropped: softmax-invariant)
  att_w      = softmax(where(mask, -inf, scores)) over S
  attended   = sum_s mm[s] * att_w[s]
  sep        = relu(si @ W_sep + b_sep + where(cnt>0, relu(havg @ W_hist + b_hist), 0))
  out[k]     = (sep[k] / max(||sep[k]||, 1e-12)) . attended
"""

import numpy as np

import concourse.bacc as bacc
import concourse.mybir as mybir
import concourse.tile as tile
from concourse.bass_utils import run_bass_kernel_spmd
from concourse.masks import make_identity

F32 = mybir.dt.float32
BF16 = mybir.dt.bfloat16
U8 = mybir.dt.uint8
I32 = mybir.dt.int32

B, S, EMB, HID, IMG, ATT, K6, HL = 64, 512, 768, 512, 2048, 512, 6, 20
NCORES = 8
BL = B // NCORES            # 8 local batches per core
NL = BL * S                 # 4096 tokens per core
BK = BL * K6                # 48 (batch, image) pairs per core

_NC_CACHE = {}


def _build_nc():
    nc = bacc.Bacc("TRN2", target_bir_lowering=False, debug=False,
                   num_devices=NCORES)
    AF = mybir.ActivationFunctionType
    OP = mybir.AluOpType

    # ---- DRAM I/O (per core) ----
    d_xt = nc.dram_tensor("xt", [EMB, NL], F32, kind="ExternalInput")
    d_sit = nc.dram_tensor("sit", [IMG, BK], F32, kind="ExternalInput")
    d_vct = nc.dram_tensor("vct", [IMG * K6 // NCORES, B], F32, kind="ExternalInput")
    d_ph = nc.dram_tensor("ph", [BK * HL, EMB], F32, kind="ExternalInput")
    d_msk = nc.dram_tensor("msk", [1, NL], U8, kind="ExternalInput")
    d_cnt = nc.dram_tensor("cnt", [2 * BK, 1], F32, kind="ExternalInput")
    d_we2h = nc.dram_tensor("we2h", [EMB, HID], F32, kind="ExternalInput")
    d_wmm = nc.dram_tensor("wmm", [2 * HID, HID], F32, kind="ExternalInput")
    d_wa1 = nc.dram_tensor("wa1", [HID, ATT], F32, kind="ExternalInput")
    d_wa2 = nc.dram_tensor("wa2", [ATT, 1], F32, kind="ExternalInput")
    d_whist = nc.dram_tensor("whist", [EMB, HID], F32, kind="ExternalInput")
    d_wsep = nc.dram_tensor("wsep", [IMG, HID], F32, kind="ExternalInput")
    d_wctx = nc.dram_tensor("wctx", [IMG * K6 // NCORES, HID], F32, kind="ExternalInput")
    d_be2h = nc.dram_tensor("be2h", [HID], F32, kind="ExternalInput")
    d_bmm = nc.dram_tensor("bmm", [HID], F32, kind="ExternalInput")
    d_ba1 = nc.dram_tensor("ba1", [ATT], F32, kind="ExternalInput")
    d_bctx = nc.dram_tensor("bctx", [HID], F32, kind="ExternalInput")
    d_bsep = nc.dram_tensor("bsep", [HID], F32, kind="ExternalInput")
    d_bhist = nc.dram_tensor("bhist", [HID], F32, kind="ExternalInput")
    d_out = nc.dram_tensor("out", [BK, 1], F32, kind="ExternalOutput")

    with tile.TileContext(nc) as tc:
        with (
            tc.tile_pool(name="const", bufs=1) as cw,
            tc.tile_pool(name="batch", bufs=2) as bp,
            tc.tile_pool(name="pbig", bufs=4, space="PSUM") as pbig,
            tc.tile_pool(name="psmall", bufs=2, space="PSUM") as psmall,
            tc.tile_pool(name="pacc", bufs=2, space="PSUM") as pacc,
        ):
            # ================= setup =================
            we2h = cw.tile([128, K6, HID], BF16)
            nc.gpsimd.dma_start(we2h[:], d_we2h.ap().rearrange("(a p) h -> p a h", p=128))
            wmm = cw.tile([128, 8, HID], BF16)
            nc.gpsimd.dma_start(wmm[:], d_wmm.ap().rearrange("(a p) h -> p a h", p=128))
            wa1 = cw.tile([128, 4, ATT], BF16)
            nc.gpsimd.dma_start(wa1[:], d_wa1.ap().rearrange("(a p) h -> p a h", p=128))
            wa2 = cw.tile([128, 4, 1], BF16)
            nc.gpsimd.dma_start(wa2[:], d_wa2.ap().rearrange("(a p) h -> p a h", p=128))
            whist = cw.tile([128, K6, HID], BF16)
            nc.gpsimd.dma_start(whist[:], d_whist.ap().rearrange("(a p) h -> p a h", p=128))
            wsep = cw.tile([128, 16, HID], BF16)
            nc.gpsimd.dma_start(wsep[:], d_wsep.ap().rearrange("(a p) h -> p a h", p=128))
            vct = cw.tile([128, 12, B], BF16)
            nc.gpsimd.dma_start(vct[:], d_vct.ap().rearrange("(a p) n -> p a n", p=128))
            wctxs = cw.tile([128, 12, HID], BF16)
            nc.gpsimd.dma_start(wctxs[:], d_wctx.ap().rearrange("(a p) h -> p a h", p=128))

            be2h = cw.tile([128, 4], F32)
            nc.sync.dma_start(be2h[:], d_be2h.ap().rearrange("(a p) -> p a", p=128))
            bmm = cw.tile([128, 4], F32)
            nc.sync.dma_start(bmm[:], d_bmm.ap().rearrange("(a p) -> p a", p=128))
            ba1 = cw.tile([128, 4], F32)
            nc.sync.dma_start(ba1[:], d_ba1.ap().rearrange("(a p) -> p a", p=128))
            brow_ctx = cw.tile([1, HID], BF16)
            nc.gpsimd.dma_start(brow_ctx[:], d_bctx.ap().rearrange("(o n) -> o n", o=1))
            brow_sep = cw.tile([1, HID], BF16)
            nc.gpsimd.dma_start(brow_sep[:], d_bsep.ap().rearrange("(o n) -> o n", o=1))
            brow_hist = cw.tile([1, HID], BF16)
            nc.gpsimd.dma_start(brow_hist[:], d_bhist.ap().rearrange("(o n) -> o n", o=1))

            msk = cw.tile([1, NL], U8)
            nc.sync.dma_start(msk[:], d_msk.ap())
            cnt = cw.tile([2 * BK, 1], F32)
            nc.sync.dma_start(cnt[:], d_cnt.ap())

            identf = cw.tile([128, 128], F32)
            make_identity(nc, identf[:])
            identb = cw.tile([128, 128], BF16)
            make_identity(nc, identb[:])
            ones_bf = cw.tile([1, 128], BF16)
            nc.gpsimd.memset(ones_bf[:], 1.0)
            a48 = cw.tile([BL, BK], BF16)
            nc.gpsimd.memset(a48[:], 0.0)
            # a48[b, n] = 1 where n // 6 == b: iota(p, n) = n//6 - p
            nc.gpsimd.affine_select(
                out=a48[:], in_=a48[:], compare_op=OP.not_equal, fill=1.0,
                base=0, pattern=[[1, BL], [0, K6]], channel_multiplier=-1)

            # ================= ctx branch (sharded + ReduceScatter) =========
            ones8th = cw.tile([1, B], BF16)
            nc.gpsimd.memset(ones8th[:], 1.0 / NCORES)
            pctx = pacc.tile([B, HID], F32, tag="acc")
            nc.tensor.matmul(pctx[:], ones8th[:], brow_ctx[:],
                             start=True, stop=False)
            for g in range(12):
                nc.tensor.matmul(pctx[:], vct[:, g, :], wctxs[:, g, :],
                                 start=False, stop=(g == 11))
            ctxpart = cw.tile([B, HID], F32)
            nc.vector.tensor_copy(ctxpart[:], pctx[:])
            with tc.tile_pool(name="dram", bufs=1, space="DRAM") as dpool:
                cc_in = dpool.tile([B, HID], F32)
                cc_out = dpool.tile([BL, HID], F32)
                nc.gpsimd.dma_start(cc_in[:], ctxpart[:])
                nc.gpsimd.collective_compute(
                    "ReduceScatter", OP.add,
                    replica_groups=[list(range(NCORES))],
                    ins=[cc_in[:]], outs=[cc_out[:]])
                ctxsum = cw.tile([BL, HID], F32)
                nc.sync.dma_start(ctxsum[:], cc_out[:])
            ctxh = cw.tile([BL, HID], BF16)
            nc.scalar.activation(ctxh[:], ctxsum[:], AF.Relu)
            ctxT = cw.tile([128, 4, BL], BF16)
            for j in range(4):
                pt = psmall.tile([128, BL], BF16, tag="small")
                nc.tensor.transpose(pt[:], ctxh[:, j * 128:(j + 1) * 128],
                                    identb[:BL, :BL])
                nc.vector.tensor_copy(ctxT[:, j, :], pt[:])
            cbiasT = cw.tile([128, 4, BL], F32)
            for mt in range(4):
                msl = slice(mt * 128, (mt + 1) * 128)
                pcb = psmall.tile([128, BL], F32, tag="small")
                for kt in range(4):
                    nc.tensor.matmul(pcb[:], wmm[:, 4 + kt, msl], ctxT[:, kt, :],
                                     start=(kt == 0), stop=(kt == 3))
                nc.vector.tensor_scalar_add(cbiasT[:, mt, :], pcb[:],
                                            bmm[:, mt:mt + 1])

            # ================= X loads (after ctx stream) =================
            xt = []
            for b in range(BL):
                nsl = slice(b * S, (b + 1) * S)
                xb = cw.tile([128, K6, S], BF16, tag=f"xt{b}")
                nc.gpsimd.dma_start(
                    xb[:], d_xt.ap()[:, nsl].rearrange("(a p) n -> p a n", p=128))
                xt.append(xb)

            # ================= per-batch main chain =================
            attT = cw.tile([128, 4, BL], F32)
            for b in range(BL):
                nsl = slice(b * S, (b + 1) * S)
                repsT = bp.tile([128, 4, S], BF16, tag="repsT")
                for mt in range(4):
                    msl = slice(mt * 128, (mt + 1) * 128)
                    pe = pbig.tile([128, S], F32, tag="big")
                    for kt in range(K6):
                        nc.tensor.matmul(pe[:], we2h[:, kt, msl], xt[b][:, kt, :],
                                         start=(kt == 0), stop=(kt == K6 - 1))
                    nc.scalar.activation(repsT[:, mt, :], pe[:], AF.Relu,
                                         bias=be2h[:, mt:mt + 1])
                mmT = bp.tile([128, 4, S], BF16, tag="mmT")
                for mt in range(4):
                    msl = slice(mt * 128, (mt + 1) * 128)
                    pm = pbig.tile([128, S], F32, tag="big")
                    for kt in range(4):
                        nc.tensor.matmul(pm[:], wmm[:, kt, msl], repsT[:, kt, :],
                                         start=(kt == 0), stop=(kt == 3))
                    nc.scalar.activation(mmT[:, mt, :], pm[:], AF.Relu,
                                         bias=cbiasT[:, mt, b:b + 1])
                aT = bp.tile([128, 4, S], BF16, tag="aT")
                for mt in range(4):
                    msl = slice(mt * 128, (mt + 1) * 128)
                    pa = pbig.tile([128, S], F32, tag="big")
                    for kt in range(4):
                        nc.tensor.matmul(pa[:], wa1[:, kt, msl], mmT[:, kt, :],
                                         start=(kt == 0), stop=(kt == 3))
                    nc.scalar.activation(aT[:, mt, :], pa[:], AF.Tanh,
                                         bias=ba1[:, mt:mt + 1])
                psc = psmall.tile([1, S], F32, tag="small")
                for kt in range(4):
                    nc.tensor.matmul(psc[:], wa2[:, kt, :], aT[:, kt, :],
                                     start=(kt == 0), stop=(kt == 3))
                # masked softmax over S (partition 0)
                mterm = bp.tile([1, S], F32, tag="mterm")
                nc.vector.tensor_scalar_mul(mterm[:], msk[0:1, nsl], -1e30)
                scm = bp.tile([1, S], F32, tag="scm")
                nc.vector.tensor_tensor(scm[:], psc[:], mterm[:], op=OP.add)
                nmax = bp.tile([1, 1], F32, tag="nmax")
                nc.vector.tensor_reduce(nmax[:], scm[:], axis=mybir.AxisListType.X,
                                        op=OP.max, negate=True)
                esc = bp.tile([1, S], F32, tag="esc")
                zsum = bp.tile([1, 1], F32, tag="zsum")
                nc.scalar.activation(esc[:], scm[:], AF.Exp,
                                     bias=nmax[:], scale=1.0, accum_out=zsum[:])
                rz = bp.tile([1, 1], F32, tag="rz")
                nc.vector.reciprocal(rz[:], zsum[:])
                attw = bp.tile([1, S], BF16, tag="attw")
                nc.vector.tensor_scalar_mul(attw[:], esc[:], rz[:])
                # broadcast att_w to 128 partitions, combine with mmT
                pwb = pbig.tile([128, S], F32, tag="big")
                nc.tensor.matmul(pwb[:], ones_bf[:], attw[:], start=True, stop=True)
                for mt in range(4):
                    scrb = bp.tile([128, S], F32, tag="scrb")
                    nc.vector.tensor_tensor(scrb[:], mmT[:, mt, :], pwb[:],
                                            op=OP.mult)
                    nc.vector.tensor_reduce(attT[:, mt, b:b + 1], scrb[:],
                                            axis=mybir.AxisListType.X, op=OP.add)

            # ================= hist + sep branch =================
            # partition q = c*48 + bk holds prev_hist[bk, :, c*384:(c+1)*384]
            phb = cw.tile([2 * BK, HL, EMB // 2], BF16)
            for c in range(2):
                nc.gpsimd.dma_start(
                    phb[c * BK:(c + 1) * BK, :, :],
                    d_ph.ap().rearrange("(bk l) (c e) -> bk c l e", l=HL, c=2)[:, c, :, :])
            iota_i = cw.tile([2 * BK, HL], I32)
            nc.gpsimd.iota(iota_i[:], pattern=[[1, HL]], base=0, channel_multiplier=0)
            iota_f = cw.tile([2 * BK, HL], F32)
            nc.vector.tensor_copy(iota_f[:], iota_i[:])
            cntc = cw.tile([2 * BK, 1], F32)
            nc.vector.tensor_scalar_max(cntc[:], cnt[:], 1.0)
            rcnt = cw.tile([2 * BK, 1], F32)
            nc.vector.reciprocal(rcnt[:], cntc[:])
            valid = cw.tile([2 * BK, HL], F32)
            nc.vector.tensor_scalar(valid[:], iota_f[:], cnt[:], None, op0=OP.is_lt)
            w96 = cw.tile([2 * BK, HL], F32)
            nc.vector.tensor_scalar_mul(w96[:], valid[:], rcnt[:])
            cp96 = cw.tile([2 * BK, 1], F32)
            nc.vector.tensor_scalar(cp96[:], cnt[:], 0.0, None, op0=OP.is_gt)
            ptc = psmall.tile([1, 2 * BK], F32, tag="small")
            nc.tensor.transpose(ptc[:], cp96[:], identf[:2 * BK, :2 * BK])
            cp48 = cw.tile([1, BK], BF16)
            nc.vector.tensor_copy(cp48[:], ptc[:, 0:BK])

            w_bc = w96[:].unsqueeze(2).broadcast_to([2 * BK, HL, EMB // 2])
            nc.vector.tensor_tensor(phb[:], phb[:], w_bc, op=OP.mult)
            havg = cw.tile([2 * BK, EMB // 2], F32)
            nc.vector.tensor_reduce(havg[:], phb[:].rearrange("p l e -> p e l"),
                                    axis=mybir.AxisListType.X, op=OP.add)
            havgT = cw.tile([128, K6, BK], BF16)
            for j in range(3):
                pt96 = psmall.tile([128, 2 * BK], F32, tag="small")
                nc.tensor.transpose(pt96[:], havg[:, j * 128:(j + 1) * 128],
                                    identf[:2 * BK, :2 * BK])
                for c in range(2):
                    nc.vector.tensor_copy(havgT[:, c * 3 + j, :],
                                          pt96[:, c * BK:(c + 1) * BK])

            php = pacc.tile([BK, HID], F32, tag="acc")
            nc.tensor.matmul(php[:], cp48[:], brow_hist[:], start=True, stop=False)
            for et in range(K6):
                nc.tensor.matmul(php[:], havgT[:, et, :], whist[:, et, :],
                                 start=False, stop=(et == K6 - 1))
            hproj = cw.tile([BK, HID], F32)
            nc.scalar.activation(hproj[:], php[:], AF.Relu)

            sit = cw.tile([128, 16, BK], BF16)
            nc.gpsimd.dma_start(sit[:], d_sit.ap().rearrange("(a p) n -> p a n", p=128))
            psep = pacc.tile([BK, HID], F32, tag="acc")
            nc.tensor.matmul(psep[:], ones_bf[:, :BK], brow_sep[:],
                             start=True, stop=False)
            for kt in range(16):
                nc.tensor.matmul(psep[:], sit[:, kt, :], wsep[:, kt, :],
                                 start=False, stop=(kt == 15))
            sep = cw.tile([BK, HID], F32)
            nc.vector.tensor_tensor(sep[:], psep[:], hproj[:], op=OP.add)
            nc.vector.tensor_scalar_max(sep[:], sep[:], 0.0)

            scr48 = cw.tile([BK, HID], F32)
            ssq = cw.tile([BK, 1], F32)
            nc.scalar.activation(scr48[:], sep[:], AF.Square, accum_out=ssq[:])
            snorm = cw.tile([BK, 1], F32)
            nc.scalar.activation(snorm[:], ssq[:], AF.Sqrt)
            snormc = cw.tile([BK, 1], F32)
            nc.vector.tensor_scalar_max(snormc[:], snorm[:], 1e-12)
            rnorm = cw.tile([BK, 1], F32)
            nc.vector.reciprocal(rnorm[:], snormc[:])

            # ================= finale =================
            attended = cw.tile([BL, HID], BF16)
            for mt in range(4):
                pt8 = psmall.tile([BL, 128], F32, tag="small")
                nc.tensor.transpose(pt8[:], attT[:, mt, :], identf[:, :])
                nc.vector.tensor_copy(attended[:, mt * 128:(mt + 1) * 128], pt8[:])
            pa48 = pacc.tile([BK, HID], F32, tag="acc")
            nc.tensor.matmul(pa48[:], a48[:], attended[:], start=True, stop=True)
            scr48b = cw.tile([BK, HID], F32)
            dotraw = cw.tile([BK, 1], F32)
            nc.vector.tensor_tensor(scr48b[:], sep[:], pa48[:], op=OP.mult)
            nc.vector.tensor_reduce(dotraw[:], scr48b[:],
                                    axis=mybir.AxisListType.X, op=OP.add)
            dotf = cw.tile([BK, 1], F32)
            nc.vector.tensor_scalar_mul(dotf[:], dotraw[:], rnorm[:])
            nc.sync.dma_start(d_out.ap(), dotf[:])

    nc.compile()
    return nc


def _get_nc():
    if "nc" not in _NC_CACHE:
        _NC_CACHE["nc"] = _build_nc()
    return _NC_CACHE["nc"]


def _make_in_maps(inputs):
    reps = np.asarray(inputs["representations"], dtype=np.float32)
    si = np.asarray(inputs["separate_images"], dtype=np.float32)
    vc = np.asarray(inputs["visual_context"], dtype=np.float32)
    ph = np.asarray(inputs["prev_hist"], dtype=np.float32)
    cnts = np.asarray(inputs["hist_counts"]).astype(np.float32)
    msks = np.asarray(inputs["masks"]).astype(np.uint8)

    shared = {
        "we2h": np.ascontiguousarray(inputs["W_e2h"], dtype=np.float32),
        "wmm": np.ascontiguousarray(inputs["W_mm"], dtype=np.float32),
        "wa1": np.ascontiguousarray(inputs["W_a1"], dtype=np.float32),
        "wa2": np.ascontiguousarray(inputs["W_a2"], dtype=np.float32).reshape(ATT, 1),
        "whist": np.ascontiguousarray(inputs["W_hist"], dtype=np.float32),
        "wsep": np.ascontiguousarray(inputs["W_sep"], dtype=np.float32),
        "be2h": np.ascontiguousarray(inputs["b_e2h"], dtype=np.float32),
        "bmm": np.ascontiguousarray(inputs["b_mm"], dtype=np.float32),
        "ba1": np.ascontiguousarray(inputs["b_a1"], dtype=np.float32),
        "bctx": np.ascontiguousarray(inputs["b_ctx"], dtype=np.float32),
        "bsep": np.ascontiguousarray(inputs["b_sep"], dtype=np.float32),
        "bhist": np.ascontiguousarray(inputs["b_hist"], dtype=np.float32),
    }
    vcT_full = np.ascontiguousarray(vc.T)
    wctx_full = np.ascontiguousarray(inputs["W_ctx"], dtype=np.float32)
    in_maps = []
    for c in range(NCORES):
        bs = slice(c * BL, (c + 1) * BL)
        m = dict(shared)
        m["xt"] = np.ascontiguousarray(
            reps[bs].transpose(2, 0, 1).reshape(EMB, NL))
        m["sit"] = np.ascontiguousarray(
            si[bs].reshape(BK, IMG).T)
        m["vct"] = np.ascontiguousarray(
            vcT_full[c * (IMG * K6 // NCORES):(c + 1) * (IMG * K6 // NCORES)])
        m["wctx"] = np.ascontiguousarray(
            wctx_full[c * (IMG * K6 // NCORES):(c + 1) * (IMG * K6 // NCORES)])
        m["ph"] = np.ascontiguousarray(ph[bs].reshape(BK * HL, EMB))
        m["msk"] = np.ascontiguousarray(
            msks[bs].reshape(1, NL))
        m["cnt"] = np.ascontiguousarray(
            np.tile(cnts[bs].reshape(BK), 2).reshape(2 * BK, 1))
        in_maps.append(m)
    return in_maps


def run(inputs, trace=False, trace_kwargs={}, run_kwargs={}):
    nc = _get_nc()
    in_maps = _make_in_maps(inputs)
    res = run_bass_kernel_spmd(nc, in_maps, core_ids=list(range(NCORES)),
                               trace=trace, trace_kwargs=trace_kwargs,
                               **run_kwargs)
    out = np.stack([res.results[c]["out"].reshape(BL, K6, 1)
                    for c in range(NCORES)])
    return out.reshape(B, K6, 1).astype(np.float32), res


def kernel(**inputs):
    out, _ = run(inputs, trace=False)
    return out
